# revision 11
# baseline (speedup 1.0000x reference)
"""ConvMod3d (StyleGAN-style modulated 3x3x3 conv, N=4 groups) on 8 trn2 cores.

Sharding: 8 shards = 4 samples x 2 depth-halves. Each core convolves a
25-plane input slab (64ch x 48x48) against its sample's modulated 64x64x27
weights, producing 23 output planes. Style modulation/demodulation of the
tiny weight tensor happens on host; the conv (99.8% of FLOPs) on device.

Per output plane d': 27 taps, each a [Cin=64 -> Cout=64] matmul over the
flattened 48x48 plane with a shifted read offset; invalid edge columns
(w'>=46) are computed and discarded on the host side.

PE packing: taps are fused two-per-matmul on the contraction dim via
stacked SBUF windows, all streams contract-128 (uniform contract avoids
the ~140ns contract-switch penalty; trn2 PE streams ~1 col/cycle/strip):
- W[p]  = plane p (partitions 0-63) | plane p+1 (64-127): fuses the
  (kd=0,kd=1) tap pairs -> 9 contract-128 streams per output plane.
- W2[p] = plane p | plane p shifted +48 (one h row): fuses (kd=2,
  kh=0/1) -> 3 streams.
- W4[p] = plane p shifted +96 | +97: fuses (kd=2,kh=2,kw=0/1) -> 1
  stream, plus (kd2,kh2,kw2) as a zero-lower stream.
  => 14 streams/plane (vs 15 in the old kernel's zero-upper scheme).
  14 is provably minimal for this family: kd0 is only reachable from
  W[d]'s upper half, forcing all 9 W streams (kd1 rides free), and
  kd2's 9 taps pack into ceil(9/2)=5 shifted-pair streams.
Two output planes run concurrently on PE col strips (plane A accumulates
in PSUM partitions 0-63 via col group 0, plane B in 64-127 via col group
1); LDWEIGHTS of one strip hides under the other strip's matmul.
Matmuls in bf16 (fp32 PSUM accumulation). fp8 DoubleRow was evaluated
and rejected: e4m3 on both operands gives ~3e-2 expected rel err > 2e-2.

Schedule: the odd tail plane (22) runs FIRST (its windows are small, so
the PE starts ~4us after launch instead of ~12us) as 3 strip-paired
sections of 2x368 cols; groups 0-10 follow with a 3-group DMA lookahead.
All window loads are plain 2D [64 x cols] descriptors on the sync
queue, as in the proven baseline (fancier descriptors -- partition-split
3D, overlapping-stride src, gpsimd-issued, SBUF->SBUF -- crashed the
device; untangling which is future work).
"""

import time

import numpy as np
import ml_dtypes

import concourse.bacc as bacc
import concourse.bass as bass
import concourse.tile as tile
from concourse import mybir
from concourse.ap import AP
from concourse.bass_utils import run_bass_kernel_spmd

EPS = 1e-8
N, CIN, COUT = 4, 64, 64
DHW, K = 48, 3
DOUT = DHW - K + 1          # 46
HALF = DOUT // 2            # 23 output planes per core
P_IN = HALF + K - 1         # 25 input planes per core
PLANE = DHW * DHW           # 2304
PAD_COLS = 192              # tail slack so shifted reads stay in-bounds
XS_COLS = P_IN * PLANE + PAD_COLS
WCOLS = PLANE + PAD_COLS - 64   # w/w2 window columns (2432)
W4COLS = 2224               # w4 window columns (reads <= 2210)
W4LEN = 2210
PLANE_OUT = (DHW - 2) * DHW     # 2208 computed output cols (h' rows 0-45)
GROUP = 2
NGROUPS = (HALF + GROUP - 1) // GROUP   # 12 (11 pairs + solo)
CHUNKS = [(0, 512), (512, 512), (1024, 512), (1536, 512), (2048, 160)]
SOLO_CSZ = 368              # 6 chunks of 368 = 2208, strip-paired
NCORES = 8
NJ = 14                     # weight blocks per main stream
NWBLK = 17                  # 14 main + 3 solo-style (kd2,kh2,kw lower)
SOLO_JS = list(range(12)) + [14, 15, 16]

F32 = mybir.dt.float32
MM_DT = mybir.dt.bfloat16
NP_MM = np.dtype(ml_dtypes.bfloat16)

_CACHE = {}
LAST_RESULTS = None  # BassKernelResults of the most recent device run


def _build_bass():
    nc = bacc.Bacc()
    xs = nc.declare_dram_parameter("xs", [CIN, XS_COLS], MM_DT, isOutput=False)
    wt = nc.declare_dram_parameter("wt", [128, NWBLK * COUT], MM_DT, isOutput=False)
    bt = nc.declare_dram_parameter("bt", [128, 1], F32, isOutput=False)
    y = nc.declare_dram_parameter(
        "y", [NGROUPS, GROUP * 64, PLANE_OUT], MM_DT, isOutput=True)

    with tile.TileContext(nc) as tc:
        with (
            tc.tile_pool(name="const", bufs=1) as cpool,
            tc.tile_pool(name="wpool", bufs=10) as wpool,
            tc.tile_pool(name="w2pool", bufs=10) as w2pool,
            tc.tile_pool(name="w4pool", bufs=9) as w4pool,
            tc.tile_pool(name="opool", bufs=3) as opool,
            tc.tile_pool(name="ppool", bufs=8, space="PSUM") as ppool,
        ):
            wtile = cpool.tile([128, NWBLK * COUT], MM_DT)
            nc.sync.dma_start(out=wtile[:, :], in_=wt[:, :])
            btile = cpool.tile([128, 1], F32)
            nc.sync.dma_start(out=btile[:, :], in_=bt[:, :])

            windows = {}

            def load_win(fam, p, upshift, coff=0, split=False):
                # baseline-style: one plain 2D descriptor per partition half.
                # window[c] = plane p col (coff+c) upper | (coff+upshift+c)
                # lower.
                pool = {"w": wpool, "w2": w2pool, "w4": w4pool}[fam]
                cols = W4COLS if fam == "w4" else WCOLS
                xw = pool.tile([128, cols], MM_DT, tag=fam, name=fam)
                cuts = [0, 768, cols] if split else [0, cols]
                base = p * PLANE + coff
                for a, b in zip(cuts, cuts[1:]):
                    nc.sync.dma_start(out=xw[0:64, a:b],
                                      in_=xs[:, base + a:base + b])
                    nc.sync.dma_start(out=xw[64:128, a:b],
                                      in_=xs[:, base + upshift + a:
                                             base + upshift + b])
                windows[(fam, p)] = xw

            issued = set()

            def issue_unit(u):
                if u is None or u in issued:
                    return
                issued.add(u)
                if u == "solo":
                    load_win("w", HALF - 1, PLANE, split=True)
                    load_win("w2", HALF + 1, DHW, split=True)
                else:
                    for d in (2 * u, 2 * u + 1):
                        load_win("w", d, PLANE)
                        load_win("w2", d + 2, DHW)
                        load_win("w4", d + 2, 1, coff=96)

            def stream_src(j, d, c0):
                if j < 9:
                    kh, kw = divmod(j, 3)
                    return windows[("w", d)], kh * DHW + kw + c0
                if j < 12:
                    return windows[("w2", d + 2)], (j - 9) + c0
                if j == 12:
                    return windows[("w4", d + 2)], c0
                if j == 13:
                    return windows[("w4", d + 2)], 2 + c0
                # solo-style: (kd2,kh2,kw) on w2 lower half, upper zero
                return windows[("w2", d + 2)], DHW + (j - 14) + c0

            # prefetch: solo unit + 3 groups of lookahead
            for u in ("solo", 0, 1, 2):
                issue_unit(u)

            # ---- solo plane (22) first: 3 sections x 2 strip-paired chunks
            d = HALF - 1
            ot = opool.tile([128, PLANE_OUT], MM_DT, tag="ot")
            for sec in range(3):
                c0s = ((2 * sec) * SOLO_CSZ, (2 * sec + 1) * SOLO_CSZ)
                pss = [ppool.tile([128, 512], F32, tag="ps", name="ps")
                       for _ in range(2)]
                for idx, j in enumerate(SOLO_JS):
                    for ci, c0 in enumerate(c0s):
                        win, off = stream_src(j, d, c0)
                        nc.tensor.matmul(
                            pss[ci][ci * 64:(ci + 1) * 64, 0:SOLO_CSZ],
                            wtile[0:128, j * 64:(j + 1) * 64],
                            win[0:128, off:off + SOLO_CSZ],
                            start=(idx == 0),
                            stop=(idx == len(SOLO_JS) - 1),
                        )
                for ci in range(2):
                    nc.scalar.activation(
                        ot[ci * 64:(ci + 1) * 64,
                           sec * SOLO_CSZ:(sec + 1) * SOLO_CSZ],
                        pss[ci][ci * 64:(ci + 1) * 64, 0:SOLO_CSZ],
                        mybir.ActivationFunctionType.Identity,
                        bias=btile[ci * 64:(ci + 1) * 64, :],
                    )
            nc.scalar.dma_start(out=y[NGROUPS - 1, 0:128, 0:3 * SOLO_CSZ],
                                in_=ot[0:128, 0:3 * SOLO_CSZ])

            # ---- main groups 0..10
            for grp in range(NGROUPS - 1):
                issue_unit(grp + 3 if grp + 3 <= NGROUPS - 2 else None)
                dps = (2 * grp, 2 * grp + 1)
                last = grp == NGROUPS - 2
                ot = opool.tile([128, PLANE_OUT], MM_DT, tag="ot")
                for c0, csz in CHUNKS:
                    pss = [ppool.tile([128, 512], F32, tag="ps", name="ps")
                           for _ in dps]
                    for idx, j in enumerate(range(NJ)):
                        for ci, d in enumerate(dps):
                            win, off = stream_src(j, d, c0)
                            nc.tensor.matmul(
                                pss[ci][ci * 64:(ci + 1) * 64, 0:csz],
                                wtile[0:128, j * 64:(j + 1) * 64],
                                win[0:128, off:off + csz],
                                start=(idx == 0),
                                stop=(idx == NJ - 1),
                            )
                    for ci in range(2):
                        nc.scalar.activation(
                            ot[ci * 64:(ci + 1) * 64, c0:c0 + csz],
                            pss[ci][ci * 64:(ci + 1) * 64, 0:csz],
                            mybir.ActivationFunctionType.Identity,
                            bias=btile[ci * 64:(ci + 1) * 64, :],
                        )
                    if last and c0 == 1536:
                        # light tail: ship the bulk before the 160-col chunk
                        nc.scalar.dma_start(out=y[grp, 0:128, 0:2048],
                                            in_=ot[0:128, 0:2048])
                if last:
                    nc.scalar.dma_start(out=y[grp, 0:128, 2048:PLANE_OUT],
                                        in_=ot[0:128, 2048:PLANE_OUT])
                else:
                    nc.scalar.dma_start(out=y[grp, 0:128, :], in_=ot[0:128, :])
    nc.compile()
    return nc


def _prep_in_maps(x, s, style_weight, style_bias, weight, bias):
    style = s @ style_weight.T + style_bias                      # [N, Cin]
    wm = weight[None] * style[:, None, :, None, None, None]      # [N,Co,Ci,k,k,k]
    wm = wm * (1.0 / np.sqrt((wm * wm).sum(axis=(2, 3, 4, 5), keepdims=True) + EPS))
    wk = wm.transpose(0, 2, 3, 4, 5, 1)                          # [N,Ci,kd,kh,kw,Co]
    wfull = np.zeros((N, 128, NWBLK * COUT), np.float32)
    for j in range(9):
        kh, kw = divmod(j, 3)
        wfull[:, 0:64, j * 64:(j + 1) * 64] = wk[:, :, 0, kh, kw, :]
        wfull[:, 64:128, j * 64:(j + 1) * 64] = wk[:, :, 1, kh, kw, :]
    for kw in range(3):
        j = 9 + kw
        wfull[:, 0:64, j * 64:(j + 1) * 64] = wk[:, :, 2, 0, kw, :]
        wfull[:, 64:128, j * 64:(j + 1) * 64] = wk[:, :, 2, 1, kw, :]
    # j12: (kd2,kh2,kw0) upper | (kd2,kh2,kw1) lower, on W4 (+96|+97)
    wfull[:, 0:64, 12 * 64:13 * 64] = wk[:, :, 2, 2, 0, :]
    wfull[:, 64:128, 12 * 64:13 * 64] = wk[:, :, 2, 2, 1, :]
    # j13: (kd2,kh2,kw2) upper only (W4 off +2), lower zero
    wfull[:, 0:64, 13 * 64:14 * 64] = wk[:, :, 2, 2, 2, :]
    # j14-16: solo style -- (kd2,kh2,kw) on W2 lower half, upper zero
    for kw in range(3):
        j = 14 + kw
        wfull[:, 64:128, j * 64:(j + 1) * 64] = wk[:, :, 2, 2, kw, :]
    wfull = np.ascontiguousarray(wfull.astype(NP_MM))
    bt = np.ascontiguousarray(
        np.tile(bias[:, None], (2, 1)), dtype=np.float32)        # [128,1]

    in_maps = []
    for core in range(NCORES):
        n, h = divmod(core, 2)
        d0 = h * HALF
        xsl = x[n, :, d0:d0 + P_IN].reshape(CIN, P_IN * PLANE)
        xsl = np.concatenate(
            [xsl, np.zeros((CIN, PAD_COLS), np.float32)], axis=1)
        in_maps.append({
            "xs": np.ascontiguousarray(xsl.astype(NP_MM)),
            "wt": wfull[n],
            "bt": bt,
        })
    return in_maps


def _gather(results):
    y = np.empty((N, COUT, DOUT, DOUT, DOUT), np.float32)
    for core in range(NCORES):
        n, h = divmod(core, 2)
        yc = results[core]["y"]
        planes = yc.reshape(NGROUPS * GROUP, COUT, DHW - 2, DHW)[:HALF - 1]
        y[n, :, h * HALF:(h + 1) * HALF - 1] = (
            planes[:, :, :, :DOUT].transpose(1, 0, 2, 3))
        # solo plane: 6 chunks of 368; even chunks rows 0:64, odd 64:128
        yg = yc[NGROUPS - 1]
        cols = np.concatenate(
            [yg[(k % 2) * 64:(k % 2) * 64 + 64,
                (k // 2) * SOLO_CSZ:(k // 2 + 1) * SOLO_CSZ]
             for k in range(6)], axis=1).reshape(COUT, DHW - 2, DHW)
        y[n, :, (h + 1) * HALF - 1] = cols[:, :, :DOUT]
    return y


def kernel(x, s, style_weight, style_bias, weight, bias):
    global LAST_RESULTS
    x = np.asarray(x, np.float32)
    s = np.asarray(s, np.float32)
    style_weight = np.asarray(style_weight, np.float32)
    style_bias = np.asarray(style_bias, np.float32)
    weight = np.asarray(weight, np.float32)
    bias = np.asarray(bias, np.float32)

    if "nc" not in _CACHE:
        _CACHE["nc"] = _build_bass()
    in_maps = _prep_in_maps(x, s, style_weight, style_bias, weight, bias)
    res = None
    for attempt in range(3):
        try:
            res = run_bass_kernel_spmd(_CACHE["nc"], in_maps, list(range(NCORES)))
            break
        except Exception:
            if attempt == 2:
                raise
            time.sleep(30)  # transient device wedge; recovers on its own
    LAST_RESULTS = res
    return _gather(res.results)
